# revision 32
# baseline (speedup 1.0000x reference)
"""Trainium2 Bass kernel v4 for nn_CrossGraphConvolution (fully-unrolled,
3-engine-balanced design).

Math (per batch b, one NeuronCore each):
    S^T[n,m] = xn[:,n] . gn[:,m]        (cosine similarity, transposed)
    P^T = exp(S^T)                       (softmax numerator; max-subtract
                                          skipped: cosines are in [-1,1])
    o3[o,m] = sum_n xw[n,o] P^T[n,m]     (aggregation pre-projected by W,
                                          fp8 DoubleRow: 2 n-chunks/matmul)
    rows[m] = sum_n P^T[n,m]             (ones-stationary fp8 DoubleRow
                                          matmuls, 2-row output)
    y[o,m]  = LeakyReLU(o3)/rows * a + b (LeakyReLU commutes with the
                                          positive 1/rows scale; BN folded)

v4 structure (vs v3's window For_i):
  - 4 m-windows of 1024 fully unrolled: no per-window all-engine barrier
    (For_i inserts InstAllEngineBarrier per iteration), cross-window
    software pipelining, all static APs.
  - exp split across engines: most chunks on ACT (table exp -> fp8 store),
    the rest on DVE via Schraudolph-in-fp8-bit-space: one tensor_scalar
    uint8 = rne(s * 8*log2e + 56 - 0.344), bitcast-read as fp8e4m3.
    (Validated: final rel err 1.1e-3 vs 2e-2 budget.)
  - rows via DoubleRow ones-stationary matmuls (2-row output, halves the
    former plain rows cost); same moving AP as the o3 matmuls.
  - PE queue uses delayed-dependent-work order: the o3/rows matmuls of
    pair p are queued after the S^T matmuls of pair p+1, so PE never
    stalls waiting for exp.
  - Epilogue of window w (recip, lrelu, /rows, BN) is interleaved into
    window w+1's instruction streams.
"""

import sys

import numpy as np

if "/opt/trn_rl_repo" not in sys.path:
    sys.path.insert(0, "/opt/trn_rl_repo")

B, C, N, M, OUT = 8, 128, 4096, 4096, 128
NJ = N // 128           # 32 n-chunks
MW = 1024               # m-window width
NMW = M // MW           # 4 m-windows (fully unrolled)
PAIRS = NJ // 2         # 16 chunk-pairs per window
EPS_BN = 1e-5
NEG_SLOPE = 0.01

# Schraudolph exp in fp8e4m3 bit space: uint8 = rne(s*SCALE + BIAS),
# bitcast fp8e4m3.  (device store convert is RNE-saturating, measured)
SCH_SCALE = 8.0 * 1.4426950408889634
SCH_BIAS = 56.0 - 0.344

# Schraudolph-on-DVE assignment: (chunk, window) -> DVE iff selector hits.
# 12 DVE / 20 ACT chunks per window; at most one DVE consumer per visit.
def _on_dve(c, w):
    return (c % 8) in (({1, 3, 5}) if w % 2 == 0 else ({2, 4, 6}))


def _apply_bir_passes():
    """Ldweights dedup + single-wait legalization (same as v3)."""
    import json

    import concourse.bass as bass

    if getattr(bass.Bass, "_bir_passes_applied", False):
        return
    orig = bass.Bass.to_json_bytes

    def patched(self):
        bir = json.loads(orig(self))
        for fn in bir.get("functions", []):
            for blk in fn.get("blocks", []):
                insts = blk.get("instructions", [])
                last_ldw = {}
                kept = []
                for ins in insts:
                    if ins.get("opcode") == "Ldweights":
                        eng = ins.get("engine")
                        key = json.dumps(
                            [
                                ins.get("ins"),
                                ins.get("perf_mode"),
                                ins.get("is_transpose"),
                                ins.get("tile_position"),
                            ],
                            sort_keys=True,
                        )
                        ow = (ins.get("sync_info") or {}).get("on_wait") or []
                        upd = (ins.get("sync_info") or {}).get("on_update") or []
                        if last_ldw.get(eng) == key and not upd:
                            if ow:
                                kept.append(
                                    {
                                        "debug": ins.get("debug", 0),
                                        "engine": eng,
                                        "ins": [],
                                        "name": ins["name"] + "-dedup",
                                        "opcode": "NoOp",
                                        "outs": [],
                                        "sync_info": {
                                            "on_update": [],
                                            "on_wait": ow,
                                        },
                                    }
                                )
                            continue
                        last_ldw[eng] = key
                    kept.append(ins)
                new_insts = []
                for ins in kept:
                    si = ins.get("sync_info")
                    ow = (si or {}).get("on_wait") or []
                    if len(ow) > 1:
                        for k, w in enumerate(ow[:-1]):
                            new_insts.append(
                                {
                                    "debug": ins.get("debug", 0),
                                    "engine": ins["engine"],
                                    "ins": [],
                                    "name": f"{ins['name']}-w{k}",
                                    "opcode": "NoOp",
                                    "outs": [],
                                    "sync_info": {
                                        "on_update": [],
                                        "on_wait": [w],
                                    },
                                }
                            )
                        si["on_wait"] = [ow[-1]]
                    new_insts.append(ins)
                blk["instructions"] = new_insts
        return json.dumps(bir).encode()

    bass.Bass.to_json_bytes = patched
    bass.Bass._bir_passes_applied = True


def _ap3(sl, t_stride, t_n, m_stride, m_n):
    """3D AP view [partition][t][m] of a 2D tile slice (for DoubleRow)."""
    import concourse.bass as bass

    return bass.AP(
        tensor=sl.tensor,
        offset=sl.offset,
        ap=[list(sl.ap[0]), [t_stride, t_n], [m_stride, m_n]],
    )


def build_nc(repeats: int = 1, schraud: bool = True, rows_dr: bool = True):
    import concourse.bass as bass
    import concourse.tile as tile
    from concourse import mybir

    _apply_bir_passes()

    f32 = mybir.dt.float32
    bf16 = mybir.dt.bfloat16
    f8 = mybir.dt.float8e4
    u8 = mybir.dt.uint8
    ALU = mybir.AluOpType
    ACTF = mybir.ActivationFunctionType
    DR = mybir.MatmulPerfMode.DoubleRow

    nc = bass.Bass("TRN2")
    xn_d = nc.dram_tensor("xn", [C, N], bf16, kind="ExternalInput")
    gn_d = nc.dram_tensor("gn", [C, M], bf16, kind="ExternalInput")
    xw_d = nc.dram_tensor("xw", [128, NJ * OUT], f8, kind="ExternalInput")
    ab_d = nc.dram_tensor("ab", [OUT, 2], f32, kind="ExternalInput")
    y_d = nc.dram_tensor("y", [OUT, M], bf16, kind="ExternalOutput")

    with tile.TileContext(nc) as tc:
        with (
            tc.tile_pool(name="const", bufs=1) as const,
            tc.tile_pool(name="sb", bufs=1) as sb,
            tc.tile_pool(name="ep", bufs=2) as ep,
            tc.tile_pool(name="stp", bufs=2, space="PSUM") as stp,
            tc.tile_pool(name="o3p", bufs=1, space="PSUM") as o3p,
            tc.tile_pool(name="rwp", bufs=1, space="PSUM") as rwp,
        ):
            ab_sb = const.tile([OUT, 2], f32, tag="ab", name="ab_sb")
            nc.gpsimd.dma_start(out=ab_sb, in_=ab_d[:])
            # all-ones DoubleRow stationary with FULL 128 columns: the rows
            # matmul then outputs the row-sum replicated on all 128
            # partitions -- the softmax-denominator broadcast comes free.
            ones_dr = const.tile([128, 256], f8, tag="onesdr", name="ones_dr")
            nc.vector.memset(ones_dr, 1.0)
            xn_sb = sb.tile([C, N], bf16, tag="xn", name="xn_sb")
            gn_sb = sb.tile([C, M], bf16, tag="gn", name="gn_sb")
            xw_sb = sb.tile([128, NJ * OUT], f8, tag="xw", name="xw_sb")
            nc.gpsimd.dma_start(out=xn_sb, in_=xn_d[:])
            nc.gpsimd.dma_start(out=gn_sb, in_=gn_d[:])
            nc.gpsimd.dma_start(out=xw_sb, in_=xw_d[:])

            y_all = sb.tile([OUT, M], bf16, tag="yall", name="y_all")
            # P^T staging for all 4 windows (128 KB/partition): lets one
            # xn LDWEIGHTS serve a window-pair of S^T matmuls.
            pt_bufs = [
                sb.tile([128, NJ * MW], f8, tag=f"pt{i}", name=f"pt{i}")
                for i in range(NMW)
            ]

            def body():
                # per-window live state for the cross-window epilogue
                state = {}

                def emit_visit_w(w, c):
                    """S^T matmuls + consumer for chunk c of window w; the
                    xn stationary is shared (deduped) across the pair of
                    windows visited back-to-back."""
                    st = stp.tile([128, MW], f32, tag="st", name="st")
                    for h in range(2):
                        nc.tensor.matmul(
                            st[:, h * 512 : (h + 1) * 512],
                            xn_sb[:, c * 128 : (c + 1) * 128],
                            gn_sb[:, w * MW + h * 512 : w * MW + (h + 1) * 512],
                            start=True,
                            stop=True,
                        )
                    emit_consumer(w, c, st)

                def emit_consumer(w, c, st):
                    pt = pt_bufs[w]
                    out_sl = pt[:, c * MW : (c + 1) * MW]
                    if _on_dve(c, w) and schraud:
                        nc.vector.tensor_scalar(
                            out=out_sl.bitcast(u8),
                            in0=st,
                            scalar1=SCH_SCALE,
                            scalar2=SCH_BIAS,
                            op0=ALU.mult,
                            op1=ALU.add,
                        )
                    else:
                        nc.scalar.activation(out=out_sl, in_=st, func=ACTF.Exp)

                def emit_o3(w, pj):
                    """o3 DoubleRow matmuls for pair pj of window w."""
                    pt = pt_bufs[w]
                    o3 = state[w]["o3"]
                    for h in range(2):
                        nc.tensor.matmul(
                            o3[:, h * 512 : (h + 1) * 512],
                            _ap3(xw_sb[:, pj * 256 : (pj + 1) * 256], 128, 2, 1, 128),
                            _ap3(pt[:, 2 * pj * MW + h * 512 :], MW, 2, 1, 512),
                            start=pj == 0,
                            stop=pj == PAIRS - 1,
                            perf_mode=DR,
                        )

                def emit_rows(w, pairs):
                    """rows matmuls for a block of pairs of window w.

                    h-major so all MMs in the block share one deduped
                    all-ones LDWEIGHTS; accumulates rowsum broadcast to all
                    128 partitions."""
                    if rows_dr == "off":
                        return
                    pt = pt_bufs[w]
                    rw = state[w]["rw"]
                    for pj in pairs:
                        for h in range(2):
                            nc.tensor.matmul(
                                rw[h],
                                _ap3(ones_dr[:, 0:256], 128, 2, 1, 128),
                                _ap3(pt[:, 2 * pj * MW + h * 512 :], MW, 2, 1, 512),
                                start=pj == 0,
                                stop=pj == PAIRS - 1,
                                perf_mode=DR,
                            )

                # epilogue pieces for window w, injected into window w+1's
                # streams (or emitted serially for the last window)
                def ep_recip(w):
                    s = state[w]
                    s["rr"] = ep.tile([128, MW], bf16, tag="rr", name="rr")
                    with nc.allow_low_precision(
                        reason="1/rowsum in bf16: 0.4% rel err, budget 2e-2"
                    ):
                        for h in range(2):
                            nc.vector.reciprocal(
                                out=s["rr"][:, h * 512 : (h + 1) * 512],
                                in_=s["rw"][h],
                            )

                def ep_lrelu(w):
                    s = state[w]
                    o3 = s["o3"]
                    s["zt"] = ep.tile([OUT, MW], f32, tag="zt", name="zt")
                    nc.vector.tensor_scalar(
                        out=s["zt"], in0=o3, scalar1=NEG_SLOPE, scalar2=None,
                        op0=ALU.mult,
                    )
                    s["z"] = ep.tile([OUT, MW], f32, tag="z", name="z")
                    nc.vector.tensor_tensor(
                        out=s["z"], in0=o3, in1=s["zt"], op=ALU.max
                    )

                def ep_z2(w):
                    s = state[w]
                    s["z2"] = ep.tile([OUT, MW], f32, tag="z2", name="z2")
                    nc.vector.tensor_tensor(
                        out=s["z2"], in0=s["z"], in1=s["rr"], op=ALU.mult
                    )

                def ep_y(w):
                    s = state[w]
                    nc.vector.tensor_scalar(
                        out=y_all[:, w * MW : (w + 1) * MW],
                        in0=s["z2"],
                        scalar1=ab_sb[:, 0:1],
                        scalar2=ab_sb[:, 1:2],
                        op0=ALU.mult,
                        op1=ALU.add,
                    )

                for w in range(NMW):
                    state[w] = {
                        "o3": o3p.tile([OUT, MW], f32, tag="o3", name="o3"),
                        "rw": [
                            rwp.tile([128, 512], f32, tag=f"rw{h}", name=f"rw{h}")
                            for h in range(2)
                        ],
                    }
                blocks = [list(range(4 * k, 4 * k + 4)) for k in range(4)]

                # ---- phase P0: visits (w0, w1) + o3/rows(w0) ----
                P0_ROWS = {6: 0, 9: 1, 12: 2}  # slot -> w0 rows block
                for s in range(PAIRS):
                    emit_visit_w(0, 2 * s)
                    emit_visit_w(1, 2 * s)
                    if s >= 2:
                        emit_o3(0, s - 2)
                    emit_visit_w(0, 2 * s + 1)
                    emit_visit_w(1, 2 * s + 1)
                    if s in P0_ROWS:
                        emit_rows(0, blocks[P0_ROWS[s]])
                emit_o3(0, PAIRS - 2)
                emit_o3(0, PAIRS - 1)

                # ---- phase P1: visits (w2, w3) + o3(w1) + o3(w2) +
                #      rows(w0 tail, w1, w2 head) + epilogues(w0, w1) ----
                P1_DVE = {0: ("lrelu", 0), 2: ("recip", 0), 4: ("z2", 0),
                          5: ("y", 0), 9: ("lrelu", 1), 11: ("recip", 1),
                          13: ("z2", 1), 14: ("y", 1)}
                P1_ROWS = {1: (0, 3), 4: (1, 0), 6: (1, 1), 8: (1, 2),
                           10: (1, 3), 12: (2, 0), 14: (2, 1)}
                EPF = {"lrelu": ep_lrelu, "recip": ep_recip,
                       "z2": ep_z2, "y": ep_y}
                for s in range(PAIRS):
                    if s in P1_DVE:
                        op, w = P1_DVE[s]
                        EPF[op](w)
                    emit_visit_w(2, 2 * s)
                    emit_visit_w(3, 2 * s)
                    if 1 <= s <= 8:
                        emit_o3(1, 2 * (s - 1))
                        emit_o3(1, 2 * (s - 1) + 1)
                    if s >= 2:
                        emit_o3(2, s - 2)
                    emit_visit_w(2, 2 * s + 1)
                    emit_visit_w(3, 2 * s + 1)
                    if s in P1_ROWS:
                        rw_w, blk = P1_ROWS[s]
                        emit_rows(rw_w, blocks[blk])
                emit_o3(2, PAIRS - 2)
                emit_o3(2, PAIRS - 1)

                # ---- phase P2: o3/rows(w3) + rows(w2 tail) + epilogues ----
                ep_lrelu(2)
                emit_rows(2, blocks[2])
                emit_rows(2, blocks[3])
                ep_recip(2)
                for pj in range(0, 4):
                    emit_o3(3, pj)
                emit_rows(3, blocks[0])
                for pj in range(4, 8):
                    emit_o3(3, pj)
                ep_z2(2)
                emit_rows(3, blocks[1])
                for pj in range(8, 12):
                    emit_o3(3, pj)
                ep_y(2)
                emit_rows(3, blocks[2])
                for pj in range(12, 16):
                    emit_o3(3, pj)
                ep_lrelu(3)
                emit_rows(3, blocks[3])
                ep_recip(3)
                ep_z2(3)
                ep_y(3)

            if repeats == 1:
                body()
            else:
                with tc.For_i(0, repeats, 1):
                    body()
            nc.gpsimd.dma_start(out=y_d[:], in_=y_all)
    return nc


_nc_cache: dict = {}


def _prep(input, target_g, weight, gamma, beta, running_mean, running_var):
    import ml_dtypes

    x = np.asarray(input, dtype=np.float32)
    g = np.asarray(target_g, dtype=np.float32)
    w = np.asarray(weight, dtype=np.float32)
    gamma = np.asarray(gamma, dtype=np.float32).reshape(OUT)
    beta = np.asarray(beta, dtype=np.float32).reshape(OUT)
    mean = np.asarray(running_mean, dtype=np.float32).reshape(OUT)
    var = np.asarray(running_var, dtype=np.float32).reshape(OUT)

    a_sc = (gamma / np.sqrt(var + EPS_BN)).astype(np.float32)
    b_sc = (beta - mean * a_sc).astype(np.float32)
    ab = np.ascontiguousarray(np.stack([a_sc, b_sc], axis=1))

    xn = x / np.maximum(np.sqrt((x * x).sum(axis=1, keepdims=True)), 1e-12)
    gn = g / np.maximum(np.sqrt((g * g).sum(axis=1, keepdims=True)), 1e-12)
    xn16 = np.ascontiguousarray(xn.astype(ml_dtypes.bfloat16))
    gn16 = np.ascontiguousarray(gn.astype(ml_dtypes.bfloat16))

    # xw[b, p, nj*128+o] = (x[b]^T @ W)[nj*128+p, o]
    xw = np.einsum("bcn,co->bno", x, w)
    xw = xw.reshape(B, NJ, 128, OUT).transpose(0, 2, 1, 3).reshape(B, 128, NJ * OUT)
    xw8 = np.ascontiguousarray(
        np.clip(xw, -224.0, 224.0).astype(ml_dtypes.float8_e4m3)
    )
    return [
        {"xn": xn16[b], "gn": gn16[b], "xw": xw8[b], "ab": ab} for b in range(B)
    ]


def kernel(input, target_g, weight, gamma, beta, running_mean, running_var):
    from concourse.bass_utils import run_bass_kernel_spmd

    if "nc" not in _nc_cache:
        _nc_cache["nc"] = build_nc(repeats=1)
    nc = _nc_cache["nc"]
    in_maps = _prep(
        input, target_g, weight, gamma, beta, running_mean, running_var
    )
    res = run_bass_kernel_spmd(nc, in_maps, core_ids=list(range(B)))
    return np.stack([res.results[b]["y"] for b in range(B)]).astype(np.float32)


# revision 35
# speedup vs baseline: 1.3959x; 1.3959x over previous
"""Trainium2 Bass kernel v4 for nn_CrossGraphConvolution (fully-unrolled,
3-engine-balanced design).

Math (per batch b, one NeuronCore each):
    S^T[n,m] = xn[:,n] . gn[:,m]        (cosine similarity, transposed)
    P^T = exp(S^T)                       (softmax numerator; max-subtract
                                          skipped: cosines are in [-1,1])
    o3[o,m] = sum_n xw[n,o] P^T[n,m]     (aggregation pre-projected by W,
                                          fp8 DoubleRow: 2 n-chunks/matmul)
    rows[m] = sum_n P^T[n,m]             (ones-stationary fp8 DoubleRow
                                          matmuls, 2-row output)
    y[o,m]  = LeakyReLU(o3)/rows * a + b (LeakyReLU commutes with the
                                          positive 1/rows scale; BN folded)

v4 structure (vs v3's window For_i):
  - 4 m-windows of 1024 fully unrolled: no per-window all-engine barrier
    (For_i inserts InstAllEngineBarrier per iteration), cross-window
    software pipelining, all static APs.
  - exp split across engines: most chunks on ACT (table exp -> fp8 store),
    the rest on DVE via Schraudolph-in-fp8-bit-space: one tensor_scalar
    uint8 = rne(s * 8*log2e + 56 - 0.344), bitcast-read as fp8e4m3.
    (Validated: final rel err 1.1e-3 vs 2e-2 budget.)
  - rows via DoubleRow ones-stationary matmuls (2-row output, halves the
    former plain rows cost); same moving AP as the o3 matmuls.
  - PE queue uses delayed-dependent-work order: the o3/rows matmuls of
    pair p are queued after the S^T matmuls of pair p+1, so PE never
    stalls waiting for exp.
  - Epilogue of window w (recip, lrelu, /rows, BN) is interleaved into
    window w+1's instruction streams.
"""

import sys

import numpy as np

if "/opt/trn_rl_repo" not in sys.path:
    sys.path.insert(0, "/opt/trn_rl_repo")

B, C, N, M, OUT = 8, 128, 4096, 4096, 128
NJ = N // 128           # 32 n-chunks
MW = 1024               # m-window width
NMW = M // MW           # 4 m-windows (fully unrolled)
PAIRS = NJ // 2         # 16 chunk-pairs per window
EPS_BN = 1e-5
NEG_SLOPE = 0.01

# Schraudolph exp in fp8e4m3 bit space: uint8 = rne(s*SCALE + BIAS),
# bitcast fp8e4m3.  (device store convert is RNE-saturating, measured)
SCH_SCALE = 8.0 * 1.4426950408889634
SCH_BIAS = 56.0 - 0.344

# Schraudolph-on-DVE assignment: (chunk, window) -> DVE iff selector hits.
# 12 DVE / 20 ACT chunks per window; at most one DVE consumer per visit.
def _on_dve(c, w):
    return (c % 8) in (({1, 3, 5}) if w % 2 == 0 else ({2, 4, 6}))


def _apply_bir_passes():
    """Ldweights dedup + single-wait legalization (same as v3)."""
    import json

    import concourse.bass as bass

    if getattr(bass.Bass, "_bir_passes_applied", False):
        return
    orig = bass.Bass.to_json_bytes

    def patched(self):
        bir = json.loads(orig(self))
        for fn in bir.get("functions", []):
            for blk in fn.get("blocks", []):
                insts = blk.get("instructions", [])
                last_ldw = {}
                kept = []
                for ins in insts:
                    if ins.get("opcode") == "Ldweights":
                        eng = ins.get("engine")
                        key = json.dumps(
                            [
                                ins.get("ins"),
                                ins.get("perf_mode"),
                                ins.get("is_transpose"),
                                ins.get("tile_position"),
                            ],
                            sort_keys=True,
                        )
                        ow = (ins.get("sync_info") or {}).get("on_wait") or []
                        upd = (ins.get("sync_info") or {}).get("on_update") or []
                        if last_ldw.get(eng) == key and not upd:
                            if ow:
                                kept.append(
                                    {
                                        "debug": ins.get("debug", 0),
                                        "engine": eng,
                                        "ins": [],
                                        "name": ins["name"] + "-dedup",
                                        "opcode": "NoOp",
                                        "outs": [],
                                        "sync_info": {
                                            "on_update": [],
                                            "on_wait": ow,
                                        },
                                    }
                                )
                            continue
                        last_ldw[eng] = key
                    kept.append(ins)
                new_insts = []
                for ins in kept:
                    si = ins.get("sync_info")
                    ow = (si or {}).get("on_wait") or []
                    if len(ow) > 1:
                        for k, w in enumerate(ow[:-1]):
                            new_insts.append(
                                {
                                    "debug": ins.get("debug", 0),
                                    "engine": ins["engine"],
                                    "ins": [],
                                    "name": f"{ins['name']}-w{k}",
                                    "opcode": "NoOp",
                                    "outs": [],
                                    "sync_info": {
                                        "on_update": [],
                                        "on_wait": [w],
                                    },
                                }
                            )
                        si["on_wait"] = [ow[-1]]
                    new_insts.append(ins)
                blk["instructions"] = new_insts
        return json.dumps(bir).encode()

    bass.Bass.to_json_bytes = patched
    bass.Bass._bir_passes_applied = True


def _ap3(sl, t_stride, t_n, m_stride, m_n):
    """3D AP view [partition][t][m] of a 2D tile slice (for DoubleRow)."""
    import concourse.bass as bass

    return bass.AP(
        tensor=sl.tensor,
        offset=sl.offset,
        ap=[list(sl.ap[0]), [t_stride, t_n], [m_stride, m_n]],
    )


def build_nc(repeats: int = 1, schraud: bool = True, rows_dr: bool = True):
    import concourse.bass as bass
    import concourse.tile as tile
    from concourse import mybir

    _apply_bir_passes()

    f32 = mybir.dt.float32
    bf16 = mybir.dt.bfloat16
    f8 = mybir.dt.float8e4
    u8 = mybir.dt.uint8
    ALU = mybir.AluOpType
    ACTF = mybir.ActivationFunctionType
    DR = mybir.MatmulPerfMode.DoubleRow

    nc = bass.Bass("TRN2")
    xn_d = nc.dram_tensor("xn", [C, N], bf16, kind="ExternalInput")
    gn_d = nc.dram_tensor("gn", [C, M], bf16, kind="ExternalInput")
    xw_d = nc.dram_tensor("xw", [128, NJ * OUT], f8, kind="ExternalInput")
    ab_d = nc.dram_tensor("ab", [OUT, 2], f32, kind="ExternalInput")
    y_d = nc.dram_tensor("y", [OUT, M], bf16, kind="ExternalOutput")

    with tile.TileContext(nc) as tc:
        with (
            tc.tile_pool(name="const", bufs=1) as const,
            tc.tile_pool(name="sb", bufs=1) as sb,
            tc.tile_pool(name="ep", bufs=2) as ep,
            tc.tile_pool(name="stp", bufs=2, space="PSUM") as stp,
            tc.tile_pool(name="o3p", bufs=1, space="PSUM") as o3p,
            tc.tile_pool(name="rwp", bufs=1, space="PSUM") as rwp,
        ):
            ab_sb = const.tile([OUT, 2], f32, tag="ab", name="ab_sb")
            nc.gpsimd.dma_start(out=ab_sb, in_=ab_d[:])
            # all-ones DoubleRow stationary with FULL 128 columns: the rows
            # matmul then outputs the row-sum replicated on all 128
            # partitions -- the softmax-denominator broadcast comes free.
            ones_dr = const.tile([128, 256], f8, tag="onesdr", name="ones_dr")
            nc.vector.memset(ones_dr, 1.0)
            xn_sb = sb.tile([C, N], bf16, tag="xn", name="xn_sb")
            gn_sb = sb.tile([C, M], bf16, tag="gn", name="gn_sb")
            xw_sb = sb.tile([128, NJ * OUT], f8, tag="xw", name="xw_sb")
            nc.gpsimd.dma_start(out=xn_sb, in_=xn_d[:])
            nc.gpsimd.dma_start(out=gn_sb, in_=gn_d[:])
            nc.gpsimd.dma_start(out=xw_sb, in_=xw_d[:])

            y_all = sb.tile([OUT, M], bf16, tag="yall", name="y_all")
            # P^T staging for all 4 windows (128 KB/partition): lets one
            # xn LDWEIGHTS serve a window-pair of S^T matmuls.
            pt_bufs = [
                sb.tile([128, NJ * MW], f8, tag=f"pt{i}", name=f"pt{i}")
                for i in range(NMW)
            ]

            def body():
                # per-window live state for the cross-window epilogue
                state = {}

                def emit_visit_w(w, c):
                    """S^T matmuls + consumer for chunk c of window w; the
                    xn stationary is shared (deduped) across the pair of
                    windows visited back-to-back."""
                    st = stp.tile([128, MW], f32, tag="st", name="st")
                    for h in range(2):
                        nc.tensor.matmul(
                            st[:, h * 512 : (h + 1) * 512],
                            xn_sb[:, c * 128 : (c + 1) * 128],
                            gn_sb[:, w * MW + h * 512 : w * MW + (h + 1) * 512],
                            start=True,
                            stop=True,
                        )
                    emit_consumer(w, c, st)

                def emit_consumer(w, c, st):
                    pt = pt_bufs[w]
                    out_sl = pt[:, c * MW : (c + 1) * MW]
                    if _on_dve(c, w) and schraud:
                        nc.vector.tensor_scalar(
                            out=out_sl.bitcast(u8),
                            in0=st,
                            scalar1=SCH_SCALE,
                            scalar2=SCH_BIAS,
                            op0=ALU.mult,
                            op1=ALU.add,
                        )
                    else:
                        nc.scalar.activation(out=out_sl, in_=st, func=ACTF.Exp)

                def emit_o3(w, pj):
                    """o3 DoubleRow matmuls for pair pj of window w."""
                    pt = pt_bufs[w]
                    o3 = state[w]["o3"]
                    for h in range(2):
                        nc.tensor.matmul(
                            o3[:, h * 512 : (h + 1) * 512],
                            _ap3(xw_sb[:, pj * 256 : (pj + 1) * 256], 128, 2, 1, 128),
                            _ap3(pt[:, 2 * pj * MW + h * 512 :], MW, 2, 1, 512),
                            start=pj == 0,
                            stop=pj == PAIRS - 1,
                            perf_mode=DR,
                        )

                def emit_rows(w, pairs):
                    """rows matmuls for a block of pairs of window w.

                    h-major so all MMs in the block share one deduped
                    all-ones LDWEIGHTS; accumulates rowsum broadcast to all
                    128 partitions."""
                    if rows_dr == "off":
                        return
                    pt = pt_bufs[w]
                    rw = state[w]["rw"]
                    for pj in pairs:
                        for h in range(2):
                            nc.tensor.matmul(
                                rw[h],
                                _ap3(ones_dr[:, 0:256], 128, 2, 1, 128),
                                _ap3(pt[:, 2 * pj * MW + h * 512 :], MW, 2, 1, 512),
                                start=pj == 0,
                                stop=pj == PAIRS - 1,
                                perf_mode=DR,
                            )

                # epilogue pieces for window w, injected into window w+1's
                # streams (or emitted serially for the last window)
                def ep_recip(w):
                    s = state[w]
                    s["rr"] = ep.tile([128, MW], bf16, tag="rr", name="rr")
                    with nc.allow_low_precision(
                        reason="1/rowsum in bf16: 0.4% rel err, budget 2e-2"
                    ):
                        for h in range(2):
                            nc.vector.reciprocal(
                                out=s["rr"][:, h * 512 : (h + 1) * 512],
                                in_=s["rw"][h],
                            )

                def ep_lrelu(w):
                    s = state[w]
                    o3 = s["o3"]
                    s["zt"] = ep.tile([OUT, MW], f32, tag="zt", name="zt")
                    nc.vector.tensor_scalar(
                        out=s["zt"], in0=o3, scalar1=NEG_SLOPE, scalar2=None,
                        op0=ALU.mult,
                    )
                    s["z"] = ep.tile([OUT, MW], f32, tag="z", name="z")
                    nc.vector.tensor_tensor(
                        out=s["z"], in0=o3, in1=s["zt"], op=ALU.max
                    )

                def ep_z2(w):
                    s = state[w]
                    s["z2"] = ep.tile([OUT, MW], f32, tag="z2", name="z2")
                    nc.vector.tensor_tensor(
                        out=s["z2"], in0=s["z"], in1=s["rr"], op=ALU.mult
                    )

                def ep_y(w):
                    s = state[w]
                    nc.vector.tensor_scalar(
                        out=y_all[:, w * MW : (w + 1) * MW],
                        in0=s["z2"],
                        scalar1=ab_sb[:, 0:1],
                        scalar2=ab_sb[:, 1:2],
                        op0=ALU.mult,
                        op1=ALU.add,
                    )

                # windows 0-2 accumulate o3/rows in the dedicated psum pools
                # (sequential lifetimes, gated by the previous window's
                # epilogue reads); window 3 lives in the st pool's banks,
                # which are idle during phase P2 -- so the w2 and w3 chains
                # don't alias and run without serialization.
                for w in range(3):
                    state[w] = {
                        "o3": o3p.tile([OUT, MW], f32, tag="o3", name="o3"),
                        "rw": [
                            rwp.tile([128, 512], f32, tag=f"rw{h}", name=f"rw{h}")
                            for h in range(2)
                        ],
                    }
                blocks = [list(range(4 * k, 4 * k + 4)) for k in range(4)]

                # ---- phase P0: visits (w0, w1) + o3/rows(w0) ----
                P0_ROWS = {6: 0, 9: 1, 12: 2}  # slot -> w0 rows block
                for s in range(PAIRS):
                    emit_visit_w(0, 2 * s)
                    emit_visit_w(1, 2 * s)
                    if s >= 2:
                        emit_o3(0, s - 2)
                    emit_visit_w(0, 2 * s + 1)
                    emit_visit_w(1, 2 * s + 1)
                    if s in P0_ROWS:
                        emit_rows(0, blocks[P0_ROWS[s]])
                emit_o3(0, PAIRS - 2)
                emit_o3(0, PAIRS - 1)

                # ---- phase P1: visits (w2, w3) + o3(w1) +
                #      rows(w0 tail, w1) + epilogues(w0, w1) ----
                P1_DVE = {0: ("lrelu", 0), 2: ("recip", 0), 4: ("z2", 0),
                          5: ("y", 0), 9: ("lrelu", 1), 11: ("recip", 1),
                          13: ("z2", 1), 14: ("y", 1)}
                P1_ROWS = {1: (0, 3), 4: (1, 0), 6: (1, 1), 8: (1, 2),
                           10: (1, 3)}
                EPF = {"lrelu": ep_lrelu, "recip": ep_recip,
                       "z2": ep_z2, "y": ep_y}
                for s in range(PAIRS):
                    if s in P1_DVE:
                        op, w = P1_DVE[s]
                        EPF[op](w)
                    emit_visit_w(2, 2 * s)
                    emit_visit_w(3, 2 * s)
                    if 1 <= s <= 8:
                        emit_o3(1, 2 * (s - 1))
                        emit_o3(1, 2 * (s - 1) + 1)
                    emit_visit_w(2, 2 * s + 1)
                    emit_visit_w(3, 2 * s + 1)
                    if s in P1_ROWS:
                        rw_w, blk = P1_ROWS[s]
                        emit_rows(rw_w, blocks[blk])

                # ---- phase P2: o3/rows(w2) in the dedicated pools,
                #      o3/rows(w3) in the idle st-pool banks ----
                state[3] = {
                    "o3": stp.tile([OUT, MW], f32, tag="st", name="o3w3"),
                }
                rw3 = stp.tile([128, MW], f32, tag="st", name="rww3")
                state[3]["rw"] = [rw3[:, 0:512], rw3[:, 512:1024]]
                for k in range(4):
                    for pj in blocks[k]:
                        emit_o3(2, pj)
                    emit_rows(2, blocks[k])
                ep_lrelu(2)
                ep_recip(2)
                for k in range(4):
                    for pj in blocks[k]:
                        emit_o3(3, pj)
                    if k == 0:
                        ep_z2(2)
                    if k == 3:
                        ep_lrelu(3)
                    emit_rows(3, blocks[k])
                    if k == 0:
                        ep_y(2)
                ep_recip(3)
                ep_z2(3)
                ep_y(3)

            if repeats == 1:
                body()
            else:
                with tc.For_i(0, repeats, 1):
                    body()
            nc.gpsimd.dma_start(out=y_d[:], in_=y_all)
    return nc


_nc_cache: dict = {}


def _prep(input, target_g, weight, gamma, beta, running_mean, running_var):
    import ml_dtypes

    x = np.asarray(input, dtype=np.float32)
    g = np.asarray(target_g, dtype=np.float32)
    w = np.asarray(weight, dtype=np.float32)
    gamma = np.asarray(gamma, dtype=np.float32).reshape(OUT)
    beta = np.asarray(beta, dtype=np.float32).reshape(OUT)
    mean = np.asarray(running_mean, dtype=np.float32).reshape(OUT)
    var = np.asarray(running_var, dtype=np.float32).reshape(OUT)

    a_sc = (gamma / np.sqrt(var + EPS_BN)).astype(np.float32)
    b_sc = (beta - mean * a_sc).astype(np.float32)
    ab = np.ascontiguousarray(np.stack([a_sc, b_sc], axis=1))

    xn = x / np.maximum(np.sqrt((x * x).sum(axis=1, keepdims=True)), 1e-12)
    gn = g / np.maximum(np.sqrt((g * g).sum(axis=1, keepdims=True)), 1e-12)
    xn16 = np.ascontiguousarray(xn.astype(ml_dtypes.bfloat16))
    gn16 = np.ascontiguousarray(gn.astype(ml_dtypes.bfloat16))

    # xw[b, p, nj*128+o] = (x[b]^T @ W)[nj*128+p, o]
    xw = np.einsum("bcn,co->bno", x, w)
    xw = xw.reshape(B, NJ, 128, OUT).transpose(0, 2, 1, 3).reshape(B, 128, NJ * OUT)
    xw8 = np.ascontiguousarray(
        np.clip(xw, -224.0, 224.0).astype(ml_dtypes.float8_e4m3)
    )
    return [
        {"xn": xn16[b], "gn": gn16[b], "xw": xw8[b], "ab": ab} for b in range(B)
    ]


def kernel(input, target_g, weight, gamma, beta, running_mean, running_var):
    from concourse.bass_utils import run_bass_kernel_spmd

    if "nc" not in _nc_cache:
        _nc_cache["nc"] = build_nc(repeats=1)
    nc = _nc_cache["nc"]
    in_maps = _prep(
        input, target_g, weight, gamma, beta, running_mean, running_var
    )
    res = run_bass_kernel_spmd(nc, in_maps, core_ids=list(range(B)))
    return np.stack([res.results[b]["y"] for b in range(B)]).astype(np.float32)
